# revision 38
# baseline (speedup 1.0000x reference)
"""Trainium2 Bass kernel for nn_BDLModel (gnn_message_passing).

Strategy (8 NeuronCores, SPMD):
  - Nodes sharded contiguously across cores (3750/core); edges partitioned by
    dst owner and sorted by dst; per dst-tile (128 nodes) edge lists padded to
    chunks of 128 edges.
  - Mean aggregation: AllGather the activation table to DRAM (fp8 e4m3 for
    the z tables, bf16 for the y table; the residual stream itself stays
    bf16 via dual-written local chunks), then per dst-tile dma_gathers
    (4 SWDGE queues) pull all
    source rows ([128, CK, W] edge-major), and per 128-edge chunk a one-hot
    selection matrix S[e,d] (generated on-chip by comparing an iota row tile
    against per-edge dst-local ids) is the stationary matmul operand:
    psum[d, :] += S^T @ Zgathered. 1/deg is applied during PSUM evacuation
    on the scalar engine.
  - The AllGather is chunked 2-ways along the local node range (one Shared
    table per chunk, edges split by src chunk) so each chunk's collective
    fires as soon as its 15 tiles are written, overlapping the remaining
    tile computation; gathers depend only on their own chunk's collective.
  - The Householder orthogonalization for D=2 has the closed form
    Q = [[c, s], [-s, c]], c=(a^2-1)/(1+a^2), s=2a/(1+a^2) where a is the
    strict-lower entry (column 2 of each 4-wide block of nr). Only the
    2::4 columns of enc_w2 are ever needed.
  - All activations node-major [128 nodes, W]; X@W matmuls use PE-transposed
    activation chunks as the stationary operand with natural [K, M] weight
    slices streaming, so outputs stay node-major.
  - Biases that are identically zero (and LN gains identically one) in the
    actual inputs are folded out of the program at build time (cache-keyed).

Self-contained: hardcodes shapes, only imports numpy + the concourse stack.
"""

import math
import os
import sys
from dataclasses import dataclass

import numpy as np

for _p in ("/opt/trn_rl_repo", "/root/.axon_site/_ro/trn_rl_repo"):
    if os.path.isdir(_p) and _p not in sys.path:
        sys.path.insert(0, _p)

import ml_dtypes  # noqa: E402

BF16 = ml_dtypes.bfloat16


@dataclass(frozen=True)
class Cfg:
    N: int = 30000
    E: int = 480000
    HID: int = 256
    NB: int = 128
    D: int = 2
    NL: int = 2
    NSAGE: int = 5
    OUT: int = 5
    NC: int = 8
    EPS: float = 1e-5
    TCH: int = 15  # tiles per AllGather chunk
    NCHK: int = 2  # AllGather chunks

    @property
    def SW(self):
        return self.D * self.D * self.NB

    @property
    def NLOC(self):
        return self.N // self.NC

    @property
    def NT(self):
        return (self.NLOC + 127) // 128

    @property
    def CH_ROWS(self):
        rows = []
        for c in range(self.NCHK):
            lo = c * self.TCH * 128
            hi = min((c + 1) * self.TCH * 128, self.NLOC)
            rows.append(hi - lo)
        return rows

    @property
    def CH_OFF_G(self):
        # global row offset of each chunk in the permuted table
        offs, acc = [], 0
        for r in self.CH_ROWS:
            offs.append(acc)
            acc += self.NC * r
        return offs


CFG = Cfg()

# names whose values may be folded out of the program when trivial
_ZERO_FOLD = [
    "b_in", "se_in_b", "sage_b1", "sage_b2", "se_out_b", "enc_b1", "enc_b2",
    "ln_b", "bdl_b1", "bdl_b2", "out_ln_b", "b_out",
]
_ONE_FOLD = ["ln_g", "out_ln_g"]


# ---------------------------------------------------------------- host prep


def _prep_rhs(w):
    """[K, M] -> [128, (K//128)*M] so slice kc -> [:, kc*M:(kc+1)*M] = W[kc]."""
    k, m = w.shape
    assert k % 128 == 0
    kc = k // 128
    return np.ascontiguousarray(
        w.reshape(kc, 128, m).transpose(1, 0, 2).reshape(128, kc * m)
    ).astype(BF16)


def _prep_bias(b):
    return np.ascontiguousarray(np.tile(np.asarray(b, np.float32).reshape(1, -1), (128, 1)))


def _prep_edges(cfg: Cfg, edge_index):
    """Partition edges by dst owner and src chunk; per (dst-tile, src-chunk)
    padded chunk schedule against the per-chunk AllGather tables."""
    src = np.asarray(edge_index[0], np.int64)
    dst = np.asarray(edge_index[1], np.int64)
    deg = np.bincount(dst, minlength=cfg.N).astype(np.float64)
    rdeg_full = (1.0 / np.maximum(deg, 1.0)).astype(np.float32)

    CH_ROWS = cfg.CH_ROWS
    # src -> (chunk, row within that chunk's table)
    s_core = src // cfg.NLOC
    s_loc = src % cfg.NLOC
    s_ch = np.minimum(s_loc // (cfg.TCH * 128), cfg.NCHK - 1)
    rows_arr = np.asarray(CH_ROWS)
    s_row = s_core * rows_arr[s_ch] + (s_loc - s_ch * cfg.TCH * 128)

    per_core = []
    ckc = [1] * cfg.NCHK
    for c in range(cfg.NC):
        lo, hi = c * cfg.NLOC, (c + 1) * cfg.NLOC
        m = (dst >= lo) & (dst < hi)
        chunks = []
        for ci in range(cfg.NCHK):
            mm = m & (s_ch == ci)
            s_c = s_row[mm]
            d_c = dst[mm] - lo
            order = np.argsort(d_c, kind="stable")
            s_c, d_c = s_c[order], d_c[order]
            bounds = np.searchsorted(d_c, np.arange(cfg.NT + 1) * 128)
            cnts = bounds[1:] - bounds[:-1]
            ck_c = max(1, int(math.ceil(cnts.max() / 128))) if len(s_c) else 1
            ckc[ci] = max(ckc[ci], ck_c)
            chunks.append((s_c, d_c, bounds))
        per_core.append(chunks)

    CKC = tuple(ckc)
    outs = []
    for c in range(cfg.NC):
        od = dict()
        for ci in range(cfg.NCHK):
            CK = CKC[ci]
            s_c, d_c, bounds = per_core[c][ci]
            idx16 = np.zeros((128, cfg.NT * CK * 8), np.int16)
            ids = np.full((128, cfg.NT * CK), 255.0, BF16)
            for t in range(cfg.NT):
                b0, b1 = bounds[t], bounds[t + 1]
                n = b1 - b0
                if n == 0:
                    continue
                i = np.arange(n)
                # gather order: unwrapped[i] = idx16[i%16, i//16] (replicated x8)
                col = t * CK * 8 + i // 16
                row = i % 16
                for g in range(8):
                    idx16[row + 16 * g, col] = s_c[b0:b1]
                ids[i % 128, t * CK + i // 128] = (d_c[b0:b1] - t * 128).astype(BF16)
            od[f"idx16_{ci}"] = idx16
            od[f"ids_f{ci}"] = ids
        rdeg = np.ones((128, cfg.NT), np.float32)
        nval = cfg.NLOC
        rfull = rdeg_full[c * cfg.NLOC : (c + 1) * cfg.NLOC]
        for t in range(cfg.NT):
            r0 = t * 128
            nr = min(128, nval - r0)
            rdeg[:nr, t] = rfull[r0 : r0 + nr]
        od["rdeg"] = rdeg
        outs.append(od)
    return CKC, outs


def _prep_inputs(cfg: Cfg, inputs):
    """Build the per-core in_maps. Returns (CKC, fold, in_maps)."""
    f32 = np.float32
    x = np.asarray(inputs["x"], f32)
    CKC, edge_outs = _prep_edges(cfg, np.asarray(inputs["edge_index"]))

    g = lambda k: np.asarray(inputs[k], f32)

    fold = frozenset(
        [k for k in _ZERO_FOLD if np.all(g(k) == 0.0)]
        + [k for k in _ONE_FOLD if np.all(g(k) == 1.0)]
    )

    shared = {
        "w_in_r": _prep_rhs(g("w_in")),
        "b_in_bc": _prep_bias(g("b_in")),
        "w_si_r": _prep_rhs(g("se_in_w")),
        "b_si_bc": _prep_bias(g("se_in_b")),
        "w_s1_r": np.concatenate([_prep_rhs(g("sage_w1")[i]) for i in range(cfg.NSAGE)], axis=1),
        "b_s1_bc": np.concatenate([_prep_bias(g("sage_b1")[i]) for i in range(cfg.NSAGE)], axis=1),
        "w_s2_r": np.concatenate([_prep_rhs(g("sage_w2")[i]) for i in range(cfg.NSAGE)], axis=1),
        "b_s2_bc": np.concatenate([_prep_bias(g("sage_b2")[i]) for i in range(cfg.NSAGE)], axis=1),
        "w_so_r": _prep_rhs(g("se_out_w")),
        "b_so_bc": _prep_bias(g("se_out_b")),
        "w_e1_r": np.concatenate([_prep_rhs(g("enc_w1")[k]) for k in range(cfg.NL)], axis=1),
        "b_e1_bc": np.concatenate([_prep_bias(g("enc_b1")[k]) for k in range(cfg.NL)], axis=1),
        "w_e2_r": np.concatenate(
            [_prep_rhs(np.ascontiguousarray(g("enc_w2")[k][:, 2::4])) for k in range(cfg.NL)], axis=1
        ),
        "b_e2_bc": np.concatenate([_prep_bias(g("enc_b2")[k][2::4]) for k in range(cfg.NL)], axis=1),
        "ln_g_bc": np.concatenate([_prep_bias(g("ln_g")[k]) for k in range(cfg.NL)], axis=1),
        "ln_b_bc": np.concatenate([_prep_bias(g("ln_b")[k]) for k in range(cfg.NL)], axis=1),
        "w_b1_r": np.concatenate([_prep_rhs(g("bdl_w1")[k]) for k in range(cfg.NL)], axis=1),
        "b_b1_bc": np.concatenate([_prep_bias(g("bdl_b1")[k]) for k in range(cfg.NL)], axis=1),
        "w_b2_r": np.concatenate([_prep_rhs(g("bdl_w2")[k]) for k in range(cfg.NL)], axis=1),
        "b_b2_bc": np.concatenate([_prep_bias(g("bdl_b2")[k]) for k in range(cfg.NL)], axis=1),
        "oln_g_bc": _prep_bias(g("out_ln_g")),
        "oln_b_bc": _prep_bias(g("out_ln_b")),
        "w_o_r": _prep_rhs(g("w_out")),
        "b_o_bc": _prep_bias(g("b_out")),
        "ident_f": np.eye(128, dtype=f32),
        "ident_b": np.eye(128, dtype=BF16),
        "iota_f": np.tile(np.arange(128), (128, 1)).astype(BF16),
    }

    in_maps = []
    for c in range(cfg.NC):
        m = dict(shared)
        m["x_c"] = np.ascontiguousarray(x[c * cfg.NLOC : (c + 1) * cfg.NLOC])
        m.update(edge_outs[c])
        in_maps.append(m)
    return CKC, fold, in_maps


# ---------------------------------------------------------------- builder


def build_program(cfg: Cfg, CKC: tuple, fold: frozenset):
    from concourse import bacc, mybir
    import concourse.tile as tile

    f32 = mybir.dt.float32
    bf16 = mybir.dt.bfloat16
    fp8 = mybir.dt.float8e4
    i16 = mybir.dt.int16
    ALU = mybir.AluOpType
    AX = mybir.AxisListType
    ACT = mybir.ActivationFunctionType

    NT, NLOC, HID, SW = cfg.NT, cfg.NLOC, cfg.HID, cfg.SW
    NCHK, TCH = cfg.NCHK, cfg.TCH
    CH_ROWS, CH_OFF_G = cfg.CH_ROWS, cfg.CH_OFF_G
    NQ = 4  # swdge queues

    nc = bacc.Bacc(
        "TRN2",
        target_bir_lowering=False,
        debug=False,
        enable_asserts=False,
        num_devices=cfg.NC,
        num_swdge_queues=NQ,
    )
    rg = [list(range(cfg.NC))]

    # ---- external I/O
    d_x = nc.dram_tensor("x_c", [NLOC, HID], f32, kind="ExternalInput").ap()
    d_idx = [
        nc.dram_tensor(f"idx16_{c}", [128, NT * CKC[c] * 8], i16, kind="ExternalInput").ap()
        for c in range(NCHK)
    ]
    CKT = sum(CKC)
    d_ids = [
        nc.dram_tensor(f"ids_f{c}", [128, NT * CKC[c]], bf16, kind="ExternalInput").ap()
        for c in range(NCHK)
    ]
    d_rdeg = nc.dram_tensor("rdeg", [128, NT], f32, kind="ExternalInput").ap()

    def din(name, shape, dt):
        return nc.dram_tensor(name, shape, dt, kind="ExternalInput").ap()

    NS, NL = cfg.NSAGE, cfg.NL
    d_w_in = din("w_in_r", [128, 2 * HID], bf16)
    d_w_si = din("w_si_r", [128, 2 * SW], bf16)
    d_w_s1 = din("w_s1_r", [128, NS * 8 * SW], bf16)
    d_w_s2 = din("w_s2_r", [128, NS * 4 * SW], bf16)
    d_w_so = din("w_so_r", [128, 4 * SW], bf16)
    d_w_e1 = din("w_e1_r", [128, NL * 4 * SW], bf16)
    d_w_e2 = din("w_e2_r", [128, NL * 4 * 128], bf16)
    d_w_b1 = din("w_b1_r", [128, NL * 4 * HID], bf16)
    d_w_b2 = din("w_b2_r", [128, NL * 2 * HID], bf16)
    d_w_o = din("w_o_r", [128, 2 * cfg.OUT], bf16)
    d_identf = din("ident_f", [128, 128], f32)
    d_identb = din("ident_b", [128, 128], bf16)
    d_iota = din("iota_f", [128, 128], bf16)

    # non-folded bias tensors only
    d_bias = {}

    def din_bias(key, name, shape):
        if key not in fold:
            d_bias[name] = din(name, shape, f32)

    din_bias("b_in", "b_in_bc", [128, HID])
    din_bias("se_in_b", "b_si_bc", [128, SW])
    din_bias("sage_b1", "b_s1_bc", [128, NS * SW])
    din_bias("sage_b2", "b_s2_bc", [128, NS * SW])
    din_bias("se_out_b", "b_so_bc", [128, SW])
    din_bias("enc_b1", "b_e1_bc", [128, NL * SW])
    din_bias("enc_b2", "b_e2_bc", [128, NL * 128])
    din_bias("ln_g", "ln_g_bc", [128, NL * HID])
    din_bias("ln_b", "ln_b_bc", [128, NL * HID])
    din_bias("bdl_b1", "b_b1_bc", [128, NL * HID])
    din_bias("bdl_b2", "b_b2_bc", [128, NL * HID])
    din_bias("out_ln_g", "oln_g_bc", [128, HID])
    din_bias("out_ln_b", "oln_b_bc", [128, HID])
    din_bias("b_out", "b_o_bc", [128, cfg.OUT])

    d_out = nc.dram_tensor("out", [NLOC, cfg.OUT], f32, kind="ExternalOutput").ap()

    # gather queue split: src chunk c uses queues 2c and 2c+1, splitting its
    # CKC[c] 128-row groups roughly in half per queue
    q_split = []
    for c in range(NCHK):
        half = (CKC[c] + 1) // 2
        q_split.append(((0, half), (half, CKC[c])))

    with tile.TileContext(nc) as tc:
        from contextlib import ExitStack

        ctx = ExitStack()
        pers = ctx.enter_context(tc.tile_pool(name="pers", bufs=1))
        wout = ctx.enter_context(tc.tile_pool(name="wout", bufs=1))
        wsage = ctx.enter_context(tc.tile_pool(name="wsage", bufs=1))
        work = ctx.enter_context(tc.tile_pool(name="work", bufs=2))
        sage = ctx.enter_context(tc.tile_pool(name="sage", bufs=3))
        small = ctx.enter_context(tc.tile_pool(name="small", bufs=2))
        spool = ctx.enter_context(tc.tile_pool(name="spool", bufs=2))
        zgp = ctx.enter_context(tc.tile_pool(name="zgp", bufs=3))
        psum = ctx.enter_context(tc.tile_pool(name="psum", bufs=2, space="PSUM"))
        dramA = ctx.enter_context(tc.tile_pool(name="dramA", bufs=2, space="DRAM"))
        dramL = ctx.enter_context(tc.tile_pool(name="dramL", bufs=2, space="DRAM"))

        # ---- persistent SBUF residents
        identf = pers.tile([128, 128], f32, name="identf")
        identb = pers.tile([128, 128], bf16, name="identb")
        iota = pers.tile([128, 128], bf16, name="iota")
        rdeg = pers.tile([128, NT], f32, name="rdegs")
        ids = [pers.tile([128, NT * CKC[c]], bf16, name=f"idss{c}") for c in range(NCHK)]
        idx = [pers.tile([128, NT * CKC[c] * 8], i16, name=f"idxs{c}") for c in range(NCHK)]
        nc.sync.dma_start(out=identf[:], in_=d_identf[:])
        nc.sync.dma_start(out=identb[:], in_=d_identb[:])
        nc.sync.dma_start(out=iota[:], in_=d_iota[:])
        nc.sync.dma_start(out=rdeg[:], in_=d_rdeg[:])
        for c in range(NCHK):
            nc.sync.dma_start(out=ids[c][:], in_=d_ids[c][:])
            nc.sync.dma_start(out=idx[c][:], in_=d_idx[c][:])

        negone = pers.tile([128, 1], f32, name="negone")
        nc.gpsimd.memset(negone[:], -1.0)
        epsc = pers.tile([128, 1], f32, name="epsc")
        nc.gpsimd.memset(epsc[:], cfg.EPS)

        h_t = [pers.tile([128, HID], f32, name=f"h{t}") for t in range(NT)]
        c_t = [pers.tile([128, 128], bf16, name=f"rc{t}") for t in range(NT)]
        s_t = [pers.tile([128, 128], bf16, name=f"rs{t}") for t in range(NT)]

        def load_w(pool, name, src, cols, dt):
            t = pool.tile([128, cols], dt, tag=name, name=name)
            nc.sync.dma_start(out=t[:], in_=src)
            return t

        def load_bias(pool, key, name, cols, sl=None):
            if key in fold:
                return None
            src = d_bias[name]
            ap = src[:] if sl is None else src[:, sl]
            return load_w(pool, name, ap, cols, f32)

        def rows_of(t):
            return min(128, NLOC - t * 128)

        def chunk_of(t):
            return min(t // TCH, NCHK - 1)

        # ---------- helpers ----------
        def transpose_into(dst, src_ap, nchunks, is_f32, ev="vector"):
            """dst[:, kc*128:(kc+1)*128] = src[:, kc*128:(kc+1)*128]^T (bf16 out)."""
            for kc in range(nchunks):
                if is_f32:
                    tp = psum.tile([128, 128], f32, tag="tr", name="trf")
                    nc.tensor.transpose(
                        tp[:], src_ap[:, kc * 128 : (kc + 1) * 128], identf[:]
                    )
                else:
                    tp = psum.tile([128, 128], bf16, tag="tr", name="trb")
                    nc.tensor.transpose(
                        tp[:], src_ap[:, kc * 128 : (kc + 1) * 128], identb[:]
                    )
                if ev == "vector":
                    nc.vector.tensor_copy(out=dst[:, kc * 128 : (kc + 1) * 128], in_=tp[:])
                else:
                    nc.scalar.copy(out=dst[:, kc * 128 : (kc + 1) * 128], in_=tp[:])

        def mm_acc(ps_ap, lhsT_tile, rhs_tile, kcs, m, rhs_block):
            """ps += sum_kc lhsT[:, kc]^T @ rhs[:, kc-block] (node-major out)."""
            for kc in range(kcs):
                nc.tensor.matmul(
                    ps_ap,
                    lhsT=lhsT_tile[:, kc * 128 : (kc + 1) * 128],
                    rhs=rhs_tile[:, kc * rhs_block + m[0] : kc * rhs_block + m[1]],
                    start=(kc == 0),
                    stop=(kc == kcs - 1),
                )

        def emit_ln(h_ap, g_bc, b_bc, out_ap, w):
            """LayerNorm, mostly on the scalar engine. g_bc/b_bc None if folded."""
            s1 = small.tile([128, 1], f32, tag="ln1", name="ln1")
            nc.vector.reduce_sum(out=s1[:], in_=h_ap, axis=AX.X)
            nm = small.tile([128, 1], f32, tag="ln2", name="ln2")
            nc.scalar.mul(nm[:], s1[:], -1.0 / w)
            cen = work.tile([128, w], f32, tag="lncen", name="lncen")
            nc.scalar.activation(cen[:], h_ap, ACT.Identity, bias=nm[:])
            sq = work.tile([128, w], f32, tag="lnsq", name="lnsq")
            v = small.tile([128, 1], f32, tag="ln3", name="ln3")
            nc.scalar.activation(sq[:], cen[:], ACT.Square, accum_out=v[:])
            vm = small.tile([128, 1], f32, tag="ln4", name="ln4")
            nc.scalar.activation(vm[:], v[:], ACT.Identity, bias=epsc[:], scale=1.0 / w)
            r = small.tile([128, 1], f32, tag="ln5", name="ln5")
            nc.vector.reciprocal(out=r[:], in_=vm[:])
            rs = small.tile([128, 1], f32, tag="ln6", name="ln6")
            nc.scalar.sqrt(out=rs[:], in_=r[:])
            if g_bc is None and b_bc is None:
                nc.scalar.mul(out_ap, cen[:], rs[:])
                return
            nc.scalar.mul(cen[:], cen[:], rs[:])
            if g_bc is not None and b_bc is not None:
                nc.vector.tensor_tensor(out=cen[:], in0=cen[:], in1=g_bc[:], op=ALU.mult)
                nc.vector.tensor_tensor(out=out_ap, in0=cen[:], in1=b_bc[:], op=ALU.add)
            elif g_bc is not None:
                nc.vector.tensor_tensor(out=out_ap, in0=cen[:], in1=g_bc[:], op=ALU.mult)
            else:
                nc.vector.tensor_tensor(out=out_ap, in0=cen[:], in1=b_bc[:], op=ALU.add)

        def emit_agg(tables, t, width, ps_ap, dt):
            """Gather + one-hot matmul segment sum for dst tile t into psum.
            tables: per-src-chunk AllGather tables."""
            zg = zgp.tile([128, CKT, width], dt, tag="zg", name="zg")
            zoff = 0
            for c in range(NCHK):
                CK = CKC[c]
                for qi, (c0, c1) in enumerate(q_split[c]):
                    if c1 > c0:
                        nc.gpsimd.dma_gather(
                            out_ap=zg[:, zoff + c0 : zoff + c1, :],
                            in_ap=tables[c][:],
                            idxs_ap=idx[c][:, t * CK * 8 + c0 * 8 : t * CK * 8 + c1 * 8],
                            num_idxs=(c1 - c0) * 128,
                            num_idxs_reg=(c1 - c0) * 128,
                            elem_size=width,
                            single_packet=False,
                            queue_num=2 * c + qi,
                        )
                zoff += CK
            Sall = spool.tile([128, CKT * 128], dt, tag="S", name="S")
            soff = 0
            for c in range(NCHK):
                CK = CKC[c]
                iota_bc = iota[:].rearrange("p (o f) -> p o f", o=1).to_broadcast([128, CK, 128])
                ids_bc = (
                    ids[c][:, t * CK : (t + 1) * CK]
                    .rearrange("p (c o) -> p c o", o=1)
                    .to_broadcast([128, CK, 128])
                )
                nc.vector.tensor_tensor(
                    out=Sall[:, soff * 128 : (soff + CK) * 128],
                    in0=iota_bc,
                    in1=ids_bc,
                    op=ALU.is_equal,
                )
                soff += CK
            for j in range(CKT):
                nc.tensor.matmul(
                    ps_ap,
                    lhsT=Sall[:, j * 128 : (j + 1) * 128],
                    rhs=zg[:, j, :],
                    start=(j == 0),
                    stop=(j == CKT - 1),
                )

        def alloc_loc(width, tagp, dt):
            return [
                dramL.tile([CH_ROWS[c], width], dt, tag=f"{tagp}{c}", name=f"{tagp}{c}")
                for c in range(NCHK)
            ]

        def alloc_table(width, tag, dt):
            return [
                dramA.tile(
                    [cfg.NC * CH_ROWS[c], width], dt,
                    tag=f"{tag}{c}", name=f"{tag}{c}", addr_space="Shared",
                )
                for c in range(NCHK)
            ]

        def write_z(locb, locf, tables, t, nr, z_bf):
            zf = work.tile([128, SW], fp8, tag="zf8", name="zf8")
            nc.scalar.copy(out=zf[:], in_=z_bf[:])
            write_chunk(locb, t, nr, z_bf[:nr, :])
            write_chunk(locf, t, nr, zf[:nr, :])
            maybe_collective(locf, tables, t)

        def write_chunk(locs, t, nr, src_ap):
            c = chunk_of(t)
            r0 = t * 128 - c * TCH * 128
            nc.sync.dma_start(out=locs[c][r0 : r0 + nr, :], in_=src_ap)

        def maybe_collective(locs, tables, t):
            c = chunk_of(t)
            if t == min(TCH * (c + 1), NT) - 1:
                # fire chunk c once its last tile was written
                nc.gpsimd.collective_compute(
                    "AllGather",
                    ALU.bypass,
                    replica_groups=rg,
                    ins=[locs[c][:].opt()],
                    outs=[tables[c][:].opt()],
                )

        # ================= persistent weights =================
        w_in_sb = load_w(wout, "w_in", d_w_in[:], 2 * HID, bf16)
        b_in_sb = load_bias(wout, "b_in", "b_in_bc", HID)
        w_si_sb = load_w(wout, "w_si", d_w_si[:], 2 * SW, bf16)
        b_si_sb = load_bias(wout, "se_in_b", "b_si_bc", SW)

        def gelu_from(ps_ap, bias_sb, out_ap):
            """out = gelu(ps + bias); direct scalar-engine PSUM evacuation."""
            if bias_sb is None:
                nc.scalar.activation(out=out_ap, in_=ps_ap, func=ACT.Gelu)
            else:
                pre = work.tile(
                    [128, out_ap.shape[-1]], f32, tag="tmpf", name="pre"
                )
                nc.vector.tensor_tensor(out=pre[:], in0=ps_ap, in1=bias_sb[:], op=ALU.add)
                nc.scalar.activation(out=out_ap, in_=pre[:], func=ACT.Gelu)

        # ================= phase 0: h0 = gelu(x @ w_in); z0 = gelu(h0 @ w_si) ====
        agt_next = alloc_table(SW, "agt", fp8)
        loc_prev = alloc_loc(SW, "locb", bf16); locf_prev = alloc_loc(SW, "locf", fp8)
        ph0 = tc.tile_pool(name="ph0", bufs=2)
        ph0ctx = ph0.__enter__()
        for t in range(NT):
            nr = rows_of(t)
            xt = ph0ctx.tile([128, HID], f32, tag="xt", name="xt")
            if nr < 128:
                nc.gpsimd.memset(xt[:], 0.0)
            nc.sync.dma_start(out=xt[:nr, :], in_=d_x[t * 128 : t * 128 + nr, :])
            xT = ph0ctx.tile([128, 2 * 128], bf16, tag="xT", name="xT")
            transpose_into(xT, xt[:], 2, True)
            hp = psum.tile([128, HID], f32, tag="mlp", name="hp")
            mm_acc(hp[:], xT, w_in_sb, 2, (0, HID), HID)
            gelu_from(hp[:], b_in_sb, h_t[t][:])
            # z0
            hT = ph0ctx.tile([128, 2 * 128], bf16, tag="hT0", name="hT0")
            transpose_into(hT, h_t[t][:], 2, True)
            zp = psum.tile([128, SW], f32, tag="mlp", name="zp")
            mm_acc(zp[:], hT, w_si_sb, 2, (0, SW), SW)
            z0 = sage.tile([128, SW], bf16, tag="znew", name="z0")
            gelu_from(zp[:], b_si_sb, z0[:])
            write_z(loc_prev, locf_prev, agt_next, t, nr, z0)
        ph0.__exit__(None, None, None)

        # ================= outer layers =================
        for k in range(NL):
            # ---- per-outer weights
            w_so_sb = load_w(wout, "w_so", d_w_so[:], 4 * SW, bf16)
            b_so_sb = load_bias(wout, "se_out_b", "b_so_bc", SW)
            w_e1_sb = load_w(wout, "w_e1", d_w_e1[:, k * 4 * SW : (k + 1) * 4 * SW], 4 * SW, bf16)
            b_e1_sb = load_bias(wout, "enc_b1", "b_e1_bc", SW, sl=slice(k * SW, (k + 1) * SW))
            w_e2_sb = load_w(wout, "w_e2", d_w_e2[:, k * 4 * 128 : (k + 1) * 4 * 128], 4 * 128, bf16)
            b_e2_sb = load_bias(wout, "enc_b2", "b_e2_bc", 128, sl=slice(k * 128, (k + 1) * 128))
            ln_g_sb = load_bias(wout, "ln_g", "ln_g_bc", HID, sl=slice(k * HID, (k + 1) * HID))
            ln_b_sb = load_bias(wout, "ln_b", "ln_b_bc", HID, sl=slice(k * HID, (k + 1) * HID))
            w_b1_sb = load_w(wout, "w_b1", d_w_b1[:, k * 4 * HID : (k + 1) * 4 * HID], 4 * HID, bf16)
            b_b1_sb = load_bias(wout, "bdl_b1", "b_b1_bc", HID, sl=slice(k * HID, (k + 1) * HID))
            w_b2_sb = load_w(wout, "w_b2", d_w_b2[:, k * 2 * HID : (k + 1) * 2 * HID], 2 * HID, bf16)
            b_b2_sb = load_bias(wout, "bdl_b2", "b_b2_bc", HID, sl=slice(k * HID, (k + 1) * HID))

            # ---- SAGE layers
            for i in range(cfg.NSAGE):
                agt = agt_next
                w1_sb = load_w(wsage, "w1", d_w_s1[:, i * 8 * SW : (i + 1) * 8 * SW], 8 * SW, bf16)
                b1_sb = load_bias(wsage, "sage_b1", "b_s1_bc", SW, sl=slice(i * SW, (i + 1) * SW))
                w2_sb = load_w(wsage, "w2", d_w_s2[:, i * 4 * SW : (i + 1) * 4 * SW], 4 * SW, bf16)
                b2_sb = load_bias(wsage, "sage_b2", "b_s2_bc", SW, sl=slice(i * SW, (i + 1) * SW))
                last = i == cfg.NSAGE - 1
                if not last:
                    loc_cur = alloc_loc(SW, "locb", bf16); locf_cur = alloc_loc(SW, "locf", fp8)
                    agt_next = alloc_table(SW, "agt", fp8)
                else:
                    locy = alloc_loc(HID, "locy", bf16)
                    agty = alloc_table(HID, "agty", bf16)

                for t in range(NT):
                    nr = rows_of(t)
                    # aggregation
                    aps = psum.tile([128, SW], f32, tag="agg", name="aps")
                    emit_agg(agt, t, SW, aps[:], fp8)
                    m_sb = sage.tile([128, SW], bf16, tag="msb", name="msb")
                    nc.scalar.mul(m_sb[:], aps[:], rdeg[:, t : t + 1])
                    # self rows
                    z_sb = sage.tile([128, SW], bf16, tag="zsb", name="zsb")
                    if nr < 128:
                        nc.gpsimd.memset(z_sb[:], 0.0)
                    c = chunk_of(t)
                    r0 = t * 128 - c * TCH * 128
                    nc.sync.dma_start(
                        out=z_sb[:nr, :], in_=loc_prev[c][r0 : r0 + nr, :]
                    )
                    # zc^T = [z | m]^T
                    zcT = sage.tile([128, 8 * 128], bf16, tag="zcT", name="zcT")
                    transpose_into(zcT[:, : 4 * 128], z_sb[:], 4, False)
                    transpose_into(zcT[:, 4 * 128 : 8 * 128], m_sb[:], 4, False)
                    # MLP1
                    p1p = psum.tile([128, SW], f32, tag="mlp", name="p1p")
                    mm_acc(p1p[:], zcT, w1_sb, 8, (0, SW), SW)
                    p1 = sage.tile([128, SW], bf16, tag="p1", name="p1")
                    gelu_from(p1p[:], b1_sb, p1[:])
                    p1T = sage.tile([128, 4 * 128], bf16, tag="p1T", name="p1T")
                    transpose_into(p1T, p1[:], 4, False)
                    # MLP2 + residual
                    p2p = psum.tile([128, SW], f32, tag="mlp", name="p2p")
                    mm_acc(p2p[:], p1T, w2_sb, 4, (0, SW), SW)
                    znew = sage.tile([128, SW], bf16, tag="znew", name="znew")
                    if b2_sb is None:
                        nc.vector.tensor_tensor(out=znew[:], in0=p2p[:], in1=z_sb[:], op=ALU.add)
                    else:
                        p2pre = work.tile([128, SW], f32, tag="tmpf", name="p2pre")
                        nc.vector.tensor_tensor(out=p2pre[:], in0=p2p[:], in1=b2_sb[:], op=ALU.add)
                        nc.vector.tensor_tensor(out=znew[:], in0=p2pre[:], in1=z_sb[:], op=ALU.add)
                    if not last:
                        write_z(loc_cur, locf_cur, agt_next, t, nr, znew)
                        continue

                    # ---- fused: enc path -> rotation coefs; LN(h) -> y -> locy
                    z5T = work.tile([128, 4 * 128], bf16, tag="z5T", name="z5T")
                    transpose_into(z5T, znew[:], 4, False)
                    ep = psum.tile([128, SW], f32, tag="mlp", name="ep")
                    mm_acc(ep[:], z5T, w_so_sb, 4, (0, SW), SW)
                    enc = work.tile([128, SW], bf16, tag="enc", name="enc")
                    if b_so_sb is None:
                        nc.scalar.copy(out=enc[:], in_=ep[:])
                    else:
                        nc.vector.tensor_tensor(out=enc[:], in0=ep[:], in1=b_so_sb[:], op=ALU.add)
                    encT = work.tile([128, 4 * 128], bf16, tag="encT", name="encT")
                    transpose_into(encT, enc[:], 4, False)
                    gp = psum.tile([128, SW], f32, tag="mlp", name="gp")
                    mm_acc(gp[:], encT, w_e1_sb, 4, (0, SW), SW)
                    gact = work.tile([128, SW], bf16, tag="gact", name="gact")
                    gelu_from(gp[:], b_e1_sb, gact[:])
                    gT = work.tile([128, 4 * 128], bf16, tag="gT", name="gT")
                    transpose_into(gT, gact[:], 4, False)
                    ap_ = psum.tile([128, 128], f32, tag="agg", name="ap_")
                    mm_acc(ap_[:], gT, w_e2_sb, 4, (0, 128), 128)
                    a_sb = work.tile([128, 128], f32, tag="a0", name="a_sb")
                    if b_e2_sb is None:
                        nc.scalar.copy(out=a_sb[:], in_=ap_[:])
                    else:
                        nc.vector.tensor_tensor(out=a_sb[:], in0=ap_[:], in1=b_e2_sb[:], op=ALU.add)
                    a2 = work.tile([128, 128], f32, tag="a1", name="a2")
                    nc.scalar.square(a2[:], a_sb[:])
                    rinv = work.tile([128, 128], f32, tag="a2t", name="rinv")
                    nc.vector.tensor_scalar(rinv[:], a2[:], 1.0, None, ALU.add)
                    nc.vector.reciprocal(out=rinv[:], in_=rinv[:])
                    nc.scalar.add(a2[:], a2[:], negone[:])
                    nc.vector.tensor_tensor(out=c_t[t][:], in0=a2[:], in1=rinv[:], op=ALU.mult)
                    nc.scalar.mul(a_sb[:], a_sb[:], 2.0)
                    nc.vector.tensor_tensor(out=s_t[t][:], in0=a_sb[:], in1=rinv[:], op=ALU.mult)

                    # LN(h) -> hn; y = rot(hn)
                    hn = work.tile([128, HID], bf16, tag="hn", name="hn")
                    emit_ln(h_t[t][:], ln_g_sb, ln_b_sb, hn[:], HID)
                    hn_ev = hn[:, 0:HID:2]
                    hn_od = hn[:, 1:HID:2]
                    y = work.tile([128, HID], bf16, tag="y", name="y")
                    t0 = work.tile([128, 128], f32, tag="r0", name="t0")
                    t1 = work.tile([128, 128], f32, tag="r1", name="t1")
                    nc.vector.tensor_tensor(out=t0[:], in0=c_t[t][:], in1=hn_ev, op=ALU.mult)
                    nc.vector.tensor_tensor(out=t1[:], in0=s_t[t][:], in1=hn_od, op=ALU.mult)
                    nc.vector.tensor_tensor(out=y[:, 0:HID:2], in0=t0[:], in1=t1[:], op=ALU.add)
                    nc.vector.tensor_tensor(out=t0[:], in0=c_t[t][:], in1=hn_od, op=ALU.mult)
                    nc.vector.tensor_tensor(out=t1[:], in0=s_t[t][:], in1=hn_ev, op=ALU.mult)
                    nc.vector.tensor_tensor(
                        out=y[:, 1:HID:2], in0=t0[:], in1=t1[:], op=ALU.subtract
                    )
                    write_chunk(locy, t, nr, y[:nr, :])
                    maybe_collective(locy, agty, t)

                if not last:
                    loc_prev = loc_cur

            # ---- BDL message + MLP, h update; fused z0 (k<NL-1) / output (k=NL-1)
            lastk = k == NL - 1
            if not lastk:
                agt_next = alloc_table(SW, "agt", fp8)
                loc_prev = alloc_loc(SW, "locb", bf16); locf_prev = alloc_loc(SW, "locf", fp8)
            else:
                oln_g_sb = load_bias(wout, "out_ln_g", "oln_g_bc", HID)
                oln_b_sb = load_bias(wout, "out_ln_b", "oln_b_bc", HID)
                w_o_sb = load_w(wout, "w_o", d_w_o[:], 2 * cfg.OUT, bf16)
                b_o_sb = load_bias(wout, "b_out", "b_o_bc", cfg.OUT)
            for t in range(NT):
                nr = rows_of(t)
                yps = psum.tile([128, HID], f32, tag="agg", name="yps")
                emit_agg(agty, t, HID, yps[:], bf16)
                ga = work.tile([128, HID], f32, tag="ga", name="ga")
                nc.scalar.mul(ga[:], yps[:], rdeg[:, t : t + 1])
                # hc = [LN(h) | msg]
                hc = work.tile([128, 2 * HID], bf16, tag="hc", name="hc")
                emit_ln(h_t[t][:], ln_g_sb, ln_b_sb, hc[:, :HID], HID)
                g_ev = ga[:, 0:HID:2]
                g_od = ga[:, 1:HID:2]
                t0 = work.tile([128, 128], f32, tag="r0", name="t0b")
                t1 = work.tile([128, 128], f32, tag="r1", name="t1b")
                nc.vector.tensor_tensor(out=t0[:], in0=c_t[t][:], in1=g_ev, op=ALU.mult)
                nc.vector.tensor_tensor(out=t1[:], in0=s_t[t][:], in1=g_od, op=ALU.mult)
                nc.vector.tensor_tensor(
                    out=hc[:, HID : 2 * HID : 2], in0=t0[:], in1=t1[:], op=ALU.subtract
                )
                nc.vector.tensor_tensor(out=t0[:], in0=s_t[t][:], in1=g_ev, op=ALU.mult)
                nc.vector.tensor_tensor(out=t1[:], in0=c_t[t][:], in1=g_od, op=ALU.mult)
                nc.vector.tensor_tensor(
                    out=hc[:, HID + 1 : 2 * HID : 2], in0=t0[:], in1=t1[:], op=ALU.add
                )
                hcT = work.tile([128, 4 * 128], bf16, tag="hcT", name="hcT")
                transpose_into(hcT, hc[:], 4, False)
                bp = psum.tile([128, HID], f32, tag="mlp", name="bp")
                mm_acc(bp[:], hcT, w_b1_sb, 4, (0, HID), HID)
                tb = work.tile([128, HID], bf16, tag="tb", name="tb")
                gelu_from(bp[:], b_b1_sb, tb[:])
                tbT = work.tile([128, 2 * 128], bf16, tag="tbT", name="tbT")
                transpose_into(tbT, tb[:], 2, False)
                b2p = psum.tile([128, HID], f32, tag="mlp", name="b2p")
                mm_acc(b2p[:], tbT, w_b2_sb, 2, (0, HID), HID)
                if b_b2_sb is None:
                    nc.vector.tensor_tensor(out=h_t[t][:], in0=h_t[t][:], in1=b2p[:], op=ALU.add)
                else:
                    dpre = work.tile([128, HID], f32, tag="tmpf", name="dpre")
                    nc.vector.tensor_tensor(out=dpre[:], in0=b2p[:], in1=b_b2_sb[:], op=ALU.add)
                    nc.vector.tensor_tensor(out=h_t[t][:], in0=h_t[t][:], in1=dpre[:], op=ALU.add)

                if not lastk:
                    # fused z0 for the next outer layer
                    hT = work.tile([128, 2 * 128], bf16, tag="tbT", name="hTz")
                    transpose_into(hT, h_t[t][:], 2, True)
                    zp = psum.tile([128, SW], f32, tag="mlp", name="zp2")
                    mm_acc(zp[:], hT, w_si_sb, 2, (0, SW), SW)
                    z0 = sage.tile([128, SW], bf16, tag="znew", name="z0b")
                    gelu_from(zp[:], b_si_sb, z0[:])
                    write_z(loc_prev, locf_prev, agt_next, t, nr, z0)
                else:
                    # fused final LN + output
                    hnf = work.tile([128, HID], bf16, tag="hn", name="hnf")
                    emit_ln(h_t[t][:], oln_g_sb, oln_b_sb, hnf[:], HID)
                    hnfT = work.tile([128, 2 * 128], bf16, tag="tbT", name="hnfT")
                    transpose_into(hnfT, hnf[:], 2, False)
                    op_ = psum.tile([128, cfg.OUT], f32, tag="agg", name="op_")
                    mm_acc(op_[:], hnfT, w_o_sb, 2, (0, cfg.OUT), cfg.OUT)
                    ot = work.tile([128, cfg.OUT], f32, tag="ot", name="ot")
                    if b_o_sb is None:
                        nc.scalar.copy(out=ot[:], in_=op_[:])
                    else:
                        nc.vector.tensor_tensor(out=ot[:], in0=op_[:], in1=b_o_sb[:], op=ALU.add)
                    nc.sync.dma_start(out=d_out[t * 128 : t * 128 + nr, :], in_=ot[:nr, :])

        ctx.close()

    nc.compile()
    return nc


# ---------------------------------------------------------------- runner

_CACHE = {}


def _get_program(cfg: Cfg, CK: int, fold: frozenset):
    key = (cfg, CK, fold)
    if key not in _CACHE:
        _CACHE[key] = build_program(cfg, CK, fold)
    return _CACHE[key]


def run(inputs, cfg: Cfg = CFG, trace: bool = False):
    from concourse import bass_utils

    CK, fold, in_maps = _prep_inputs(cfg, inputs)
    nc = _get_program(cfg, CK, fold)
    res = bass_utils.run_bass_kernel_spmd(
        nc, in_maps, core_ids=list(range(cfg.NC)), trace=trace
    )
    out = np.concatenate([np.asarray(res.results[c]["out"]) for c in range(cfg.NC)], axis=0)
    return out, res


def kernel(**inputs):
    out, _ = run(inputs)
    return out
